# revision 1
# baseline (speedup 1.0000x reference)
"""Trainium2 Bass kernel for nn_CGPBlock (attention block with 1x1-conv QKV).

Reference computation (per batch b):
    q = Wq @ pose + bq; k = Wk @ id + bk; v = Wv @ pose + bv     # [C, L]
    energy[i, j] = sum_c q[c, i] k[c, j]                          # [L, L]
    attn = softmax_j(energy)
    va[c, i] = sum_j v[c, j] attn[i, j]
    out = pose + gamma * va

Sharding: data-parallel over batch, B=8 batches -> 8 NeuronCores (SPMD, no
collectives). Per core: C=128 fits the partition dim exactly, L=4096.

Device algorithm (per core, all matmuls bf16 with fp32 PSUM accumulate):
  - q = WqT.T @ pose_bf + bq  -> q_sb bf16 [C, L]   (DVE adds bias, casts)
  - k = WkT.T @ id_bf + bk    -> k_sb bf16 [C, L]
  - vT[j, c] = pose_bf[:, jt].T @ WvT  -> vt_sb bf16 [128, L] (32 j-tiles;
    v bias folded into the output: va/Z + bv, since attn rows sum to 1)
  - For each i-chunk (512 cols) accumulate over 32 j-tiles:
      eT[j, i] = k_jt.T @ q_chunk          (PSUM)     -- energy transposed
      pT = exp(eT)                         (ACT, no max-sub: |E| <~ 30, safe
                                            in fp32; softmax is shift-free)
      va[c, i] += vt_jt.T @ pT             (PSUM accumulate)
      Z[1, i]  += ones.T @ pT              (PSUM accumulate, M=1)
    then out_chunk = pose' + gamma * va * (1/Z), where pose' = pose+gamma*bv.
  - Softmax-over-j needs column sums in the [j, i] layout; the ones-matmul
    provides them without any transposes.

The emission order software-pipelines PE vs ACT (va/Z matmuls for j-tile
t-2 are emitted after the energy matmul for j-tile t) because each engine
executes its instruction stream in order.
"""

import numpy as np
import ml_dtypes

import concourse.bacc as bacc
import concourse.tile as tile
from concourse import mybir
from concourse.bass_utils import run_bass_kernel_spmd

F32 = mybir.dt.float32
BF16 = mybir.dt.bfloat16
AF = mybir.ActivationFunctionType
ALU = mybir.AluOpType

B, C, L = 8, 128, 4096
CHUNK = 512                 # i-chunk width (one PSUM bank of fp32)
NCH = L // CHUNK            # 8 chunks
NJT = L // 128              # 32 j-tiles
SKEW = 2                    # software pipeline depth (PE runs ahead of ACT)

_CACHE = {}


def _build():
    nc = bacc.Bacc("TRN2", target_bir_lowering=False, debug=False, num_devices=B)

    pose_d = nc.dram_tensor("pose", [C, L], F32, kind="ExternalInput").ap()
    posebf_d = nc.dram_tensor("posebf", [C, L], BF16, kind="ExternalInput").ap()
    idbf_d = nc.dram_tensor("idbf", [C, L], BF16, kind="ExternalInput").ap()
    wt_d = nc.dram_tensor("wt", [C, 3 * C], BF16, kind="ExternalInput").ap()
    bq_d = nc.dram_tensor("bq", [C, 1], F32, kind="ExternalInput").ap()
    bk_d = nc.dram_tensor("bk", [C, 1], F32, kind="ExternalInput").ap()
    bfin_d = nc.dram_tensor("bfin", [C, 1], F32, kind="ExternalInput").ap()
    gam_d = nc.dram_tensor("gam", [C, 1], F32, kind="ExternalInput").ap()
    out_d = nc.dram_tensor("out", [C, L], F32, kind="ExternalOutput").ap()

    with tile.TileContext(nc) as tc:
        with tc.tile_pool(name="res", bufs=1) as res:
            pose_sb = res.tile([C, L], F32)
            nc.sync.dma_start(pose_sb, pose_d)
            posebf_sb = res.tile([C, L], BF16)
            nc.sync.dma_start(posebf_sb, posebf_d)
            idbf_sb = res.tile([C, L], BF16)
            nc.sync.dma_start(idbf_sb, idbf_d)
            wt_sb = res.tile([C, 3 * C], BF16)
            nc.sync.dma_start(wt_sb, wt_d)
            bq_sb = res.tile([C, 1], F32)
            nc.sync.dma_start(bq_sb, bq_d)
            bk_sb = res.tile([C, 1], F32)
            nc.sync.dma_start(bk_sb, bk_d)
            bfin_sb = res.tile([C, 1], F32)
            nc.sync.dma_start(bfin_sb, bfin_d)
            gam_sb = res.tile([C, 1], F32)
            nc.sync.dma_start(gam_sb, gam_d)
            ones_sb = res.tile([C, 1], BF16)
            nc.vector.memset(ones_sb, 1.0)

            q_sb = res.tile([C, L], BF16)
            k_sb = res.tile([C, L], BF16)
            vt_sb = res.tile([C, L], BF16)   # [j (partition), jt*128 + c]

            # pose' = pose + gamma*bv (per-partition const), reused as residual
            nc.vector.tensor_scalar_add(pose_sb, pose_sb, bfin_sb)

            wqT = wt_sb[:, 0:C]
            wkT = wt_sb[:, C:2 * C]
            wvT = wt_sb[:, 2 * C:3 * C]

            # ---- QKV convs (1x1 = channel-mixing matmuls) ----
            with tc.tile_pool(name="conv_ps", bufs=2, space="PSUM") as conv_ps:
                for ch in range(NCH):
                    sl = slice(ch * CHUNK, (ch + 1) * CHUNK)
                    qp = conv_ps.tile([C, CHUNK], F32, tag="qp")
                    nc.tensor.matmul(qp, lhsT=wqT, rhs=posebf_sb[:, sl],
                                     start=True, stop=True)
                    nc.vector.tensor_scalar_add(q_sb[:, sl], qp, bq_sb)
                    kp = conv_ps.tile([C, CHUNK], F32, tag="kp")
                    nc.tensor.matmul(kp, lhsT=wkT, rhs=idbf_sb[:, sl],
                                     start=True, stop=True)
                    nc.vector.tensor_scalar_add(k_sb[:, sl], kp, bk_sb)
                for jt in range(NJT):
                    jsl = slice(jt * 128, (jt + 1) * 128)
                    vp = conv_ps.tile([C, 128], F32, tag="vp")
                    nc.tensor.matmul(vp, lhsT=posebf_sb[:, jsl], rhs=wvT,
                                     start=True, stop=True)
                    nc.vector.tensor_copy(vt_sb[:, jsl], vp)

            # ---- attention ----
            with (
                tc.tile_pool(name="et_ps", bufs=3, space="PSUM") as et_ps,
                tc.tile_pool(name="va_ps", bufs=2, space="PSUM") as va_ps,
                tc.tile_pool(name="z_ps", bufs=2, space="PSUM") as z_ps,
                tc.tile_pool(name="pt_sb", bufs=SKEW + 2) as pt_pool,
                tc.tile_pool(name="nrm", bufs=2) as nrm,
                tc.tile_pool(name="outb", bufs=3) as outb,
                tc.tile_pool(name="dramp", bufs=2, space="DRAM") as dramp,
            ):
                for ch in range(NCH):
                    isl = slice(ch * CHUNK, (ch + 1) * CHUNK)
                    va = va_ps.tile([C, CHUNK], F32)
                    z = z_ps.tile([1, CHUNK], F32)
                    pts = {}
                    for jt in range(NJT + SKEW):
                        if jt < NJT:
                            jsl = slice(jt * 128, (jt + 1) * 128)
                            et = et_ps.tile([C, CHUNK], F32)
                            nc.tensor.matmul(et, lhsT=k_sb[:, jsl],
                                             rhs=q_sb[:, isl],
                                             start=True, stop=True)
                            pt = pt_pool.tile([C, CHUNK], BF16)
                            nc.scalar.activation(pt, et, AF.Exp)
                            pts[jt] = pt
                        lag = jt - SKEW
                        if lag >= 0:
                            pjsl = slice(lag * 128, (lag + 1) * 128)
                            pt = pts.pop(lag)
                            nc.tensor.matmul(va, lhsT=vt_sb[:, pjsl], rhs=pt,
                                             start=(lag == 0),
                                             stop=(lag == NJT - 1))
                            nc.tensor.matmul(z, lhsT=ones_sb, rhs=pt,
                                             start=(lag == 0),
                                             stop=(lag == NJT - 1))

                    rz = nrm.tile([1, CHUNK], F32, tag="rz")
                    nc.vector.reciprocal(rz, z)
                    rzd = dramp.tile([1, CHUNK], F32)
                    nc.sync.dma_start(rzd, rz)
                    rzb = nrm.tile([C, CHUNK], F32, tag="rzb")
                    nc.sync.dma_start(rzb, rzd.to_broadcast([C, CHUNK]))
                    t = nrm.tile([C, CHUNK], F32, tag="t")
                    nc.vector.tensor_mul(t, va, rzb)
                    o = outb.tile([C, CHUNK], F32)
                    nc.vector.scalar_tensor_tensor(
                        o, in0=t, scalar=gam_sb, in1=pose_sb[:, isl],
                        op0=ALU.mult, op1=ALU.add)
                    nc.sync.dma_start(out_d[:, isl], o)

    nc.compile()
    return nc


def _get_nc():
    if "nc" not in _CACHE:
        _CACHE["nc"] = _build()
    return _CACHE["nc"]


def kernel(pose_f, id_f, Wq, bq, Wk, bk, Wv, bv, gamma, **run_kwargs):
    pose_f = np.asarray(pose_f, dtype=np.float32)
    id_f = np.asarray(id_f, dtype=np.float32)
    Wq = np.asarray(Wq, dtype=np.float32)
    Wk = np.asarray(Wk, dtype=np.float32)
    Wv = np.asarray(Wv, dtype=np.float32)
    bq = np.asarray(bq, dtype=np.float32)
    bk = np.asarray(bk, dtype=np.float32)
    bv = np.asarray(bv, dtype=np.float32)
    g = float(np.asarray(gamma, dtype=np.float32).reshape(-1)[0])

    bf = ml_dtypes.bfloat16
    wt = np.concatenate([Wq.T, Wk.T, Wv.T], axis=1).astype(bf)  # [C_in, 3C]
    posebf = pose_f.astype(bf)
    idbf = id_f.astype(bf)
    bq_c = np.ascontiguousarray(bq.reshape(C, 1))
    bk_c = np.ascontiguousarray(bk.reshape(C, 1))
    bfin = np.ascontiguousarray((g * bv).reshape(C, 1).astype(np.float32))
    gam = np.full((C, 1), g, dtype=np.float32)

    in_maps = []
    for b in range(B):
        in_maps.append({
            "pose": pose_f[b],
            "posebf": posebf[b],
            "idbf": idbf[b],
            "wt": wt,
            "bq": bq_c,
            "bk": bk_c,
            "bfin": bfin,
            "gam": gam,
        })

    nc = _get_nc()
    res = run_bass_kernel_spmd(nc, in_maps, core_ids=list(range(B)), **run_kwargs)
    out = np.stack([res.results[b]["out"] for b in range(B)], axis=0)
    if run_kwargs:
        _CACHE["last_result"] = res
    return out


# revision 4
# speedup vs baseline: 1.2566x; 1.2566x over previous
"""Trainium2 Bass kernel for nn_CGPBlock (attention block with 1x1-conv QKV).

Reference computation (per batch b):
    q = Wq @ pose + bq; k = Wk @ id + bk; v = Wv @ pose + bv     # [C, L]
    energy[i, j] = sum_c q[c, i] k[c, j]                          # [L, L]
    attn = softmax_j(energy)
    va[c, i] = sum_j v[c, j] attn[i, j]
    out = pose + gamma * va

Sharding: data-parallel over batch, B=8 batches -> 8 NeuronCores (SPMD, no
collectives). Per core: C=128 fits the partition dim exactly, L=4096.

Device algorithm (per core, matmuls bf16 with fp32 PSUM accumulate):
  - q = WqT.T @ pose_bf + bq  -> q_sb bf16 [C, L]
  - k = WkT.T @ id_bf + bk    -> k_sb bf16 [C, L]
  - vT[j, c] = pose_bf[:, jt].T @ WvT -> vt_sb bf16 (v bias folded into the
    output residual, since attn rows sum to 1: va/Z + bv).
  - For each i-chunk (1024 cols), accumulate over 32 j-tiles:
      eT[j, i] = k_jt.T @ q_chunk         (PSUM, 2 x N=512 matmuls)
      pT = exp(eT)                        (one ACT op per j-tile; no max-sub:
                                           |E| < 32 so fp32 exp is safe, and
                                           softmax is shift-invariant)
      va[c, i] += vt_jt.T @ pT            (PSUM accumulate)
      Z[1, i]  += ones.T @ (pT quadsum)   (DVE pre-sums 4 j-tiles, then one
                                           M=1 matmul per quad — softmax
                                           column sums in the [j,i] layout
                                           without any transposes)
    then out = pose' + gamma * va * (1/Z), pose' = pose + gamma*bv.
  - Z is broadcast across partitions via a tiny DRAM round-trip; reciprocal
    runs on the broadcast [C, chunk] tile (1-partition DVE ops are slow).

Emission order software-pipelines PE vs ACT (va matmuls run SKEW j-tiles
behind the energy matmuls) because each engine executes in program order.
"""

import numpy as np
import ml_dtypes

import concourse.bacc as bacc
import concourse.tile as tile
from concourse import mybir
from concourse.bass_utils import run_bass_kernel_spmd

F32 = mybir.dt.float32
BF16 = mybir.dt.bfloat16
AF = mybir.ActivationFunctionType
ALU = mybir.AluOpType

B, C, L = 8, 128, 4096
CHUNK = 1024                # i-chunk width
NCH = L // CHUNK            # 4 chunks
NJT = L // 128              # 32 j-tiles
QUAD = 4                    # j-tiles pre-summed per Z matmul
SKEW = 2                    # software pipeline depth (PE runs ahead of ACT)

_CACHE = {}


def _build():
    nc = bacc.Bacc("TRN2", target_bir_lowering=False, debug=False, num_devices=B)

    pose_d = nc.dram_tensor("pose", [C, L], F32, kind="ExternalInput").ap()
    posebf_d = nc.dram_tensor("posebf", [C, L], BF16, kind="ExternalInput").ap()
    idbf_d = nc.dram_tensor("idbf", [C, L], BF16, kind="ExternalInput").ap()
    wt_d = nc.dram_tensor("wt", [C, 3 * C], BF16, kind="ExternalInput").ap()
    bq_d = nc.dram_tensor("bq", [C, 1], F32, kind="ExternalInput").ap()
    bk_d = nc.dram_tensor("bk", [C, 1], F32, kind="ExternalInput").ap()
    bfin_d = nc.dram_tensor("bfin", [C, 1], F32, kind="ExternalInput").ap()
    gam_d = nc.dram_tensor("gam", [C, 1], F32, kind="ExternalInput").ap()
    out_d = nc.dram_tensor("out", [C, L], F32, kind="ExternalOutput").ap()

    with tile.TileContext(nc) as tc:
        with tc.tile_pool(name="res", bufs=1) as res:
            wt_sb = res.tile([C, 3 * C], BF16)
            nc.sync.dma_start(wt_sb, wt_d)
            bq_sb = res.tile([C, 1], F32)
            nc.sync.dma_start(bq_sb, bq_d)
            bk_sb = res.tile([C, 1], F32)
            nc.sync.dma_start(bk_sb, bk_d)
            bfin_sb = res.tile([C, 1], F32)
            nc.sync.dma_start(bfin_sb, bfin_d)
            gam_sb = res.tile([C, 1], F32)
            nc.sync.dma_start(gam_sb, gam_d)
            ones_sb = res.tile([C, 1], BF16)
            nc.vector.memset(ones_sb, 1.0)

            pose_sb = res.tile([C, L], F32)
            posebf_sb = res.tile([C, L], BF16)
            idbf_sb = res.tile([C, L], BF16)
            q_sb = res.tile([C, L], BF16)
            k_sb = res.tile([C, L], BF16)
            vt_sb = res.tile([C, L], BF16)   # [j (partition), jt*128 + c]

            # Chunked input DMA so the QKV matmuls can start early.
            for ch in range(NCH):
                sl = slice(ch * CHUNK, (ch + 1) * CHUNK)
                nc.sync.dma_start(idbf_sb[:, sl], idbf_d[:, sl])
                nc.sync.dma_start(posebf_sb[:, sl], posebf_d[:, sl])
            for ch in range(NCH):
                sl = slice(ch * CHUNK, (ch + 1) * CHUNK)
                nc.sync.dma_start(pose_sb[:, sl], pose_d[:, sl])

            wqT = wt_sb[:, 0:C]
            wkT = wt_sb[:, C:2 * C]
            wvT = wt_sb[:, 2 * C:3 * C]

            # pose' = pose + gamma*bv (per-partition const), reused as residual
            nc.vector.tensor_scalar_add(pose_sb, pose_sb, bfin_sb)

            # ---- QKV convs (1x1 = channel-mixing matmuls) ----
            with tc.tile_pool(name="conv_ps", bufs=1, space="PSUM") as conv_ps:
                for ch in range(NCH):
                    kp = conv_ps.tile([C, CHUNK], F32, tag="kp")
                    qp = conv_ps.tile([C, CHUNK], F32, tag="qp")
                    for h in range(CHUNK // 512):
                        sl = slice(ch * CHUNK + h * 512, ch * CHUNK + (h + 1) * 512)
                        hs = slice(h * 512, (h + 1) * 512)
                        nc.tensor.matmul(kp[:, hs], lhsT=wkT, rhs=idbf_sb[:, sl],
                                         start=True, stop=True)
                        nc.tensor.matmul(qp[:, hs], lhsT=wqT, rhs=posebf_sb[:, sl],
                                         start=True, stop=True)
                    csl = slice(ch * CHUNK, (ch + 1) * CHUNK)
                    nc.vector.tensor_scalar_add(k_sb[:, csl], kp, bk_sb)
                    nc.vector.tensor_scalar_add(q_sb[:, csl], qp, bq_sb)
                    for jt in range(ch * (CHUNK // 128), (ch + 1) * (CHUNK // 128)):
                        jsl = slice(jt * 128, (jt + 1) * 128)
                        vp = conv_ps.tile([C, 128], F32, tag="vp", bufs=4)
                        nc.tensor.matmul(vp, lhsT=posebf_sb[:, jsl], rhs=wvT,
                                         start=True, stop=True)
                        nc.vector.tensor_copy(vt_sb[:, jsl], vp)

            # ---- attention ----
            with (
                tc.tile_pool(name="et_ps", bufs=2, space="PSUM") as et_ps,
                tc.tile_pool(name="va_ps", bufs=1, space="PSUM") as va_ps,
                tc.tile_pool(name="z_ps", bufs=1, space="PSUM") as z_ps,
                tc.tile_pool(name="pt_sb", bufs=SKEW + QUAD + 2) as pt_pool,
                tc.tile_pool(name="qs_sb", bufs=2) as qs_pool,
                tc.tile_pool(name="nrm", bufs=2) as nrm,
                tc.tile_pool(name="outb", bufs=2) as outb,
                tc.tile_pool(name="dramp", bufs=2, space="DRAM") as dramp,
            ):
                for ch in range(NCH):
                    i0 = ch * CHUNK
                    isl = slice(i0, i0 + CHUNK)
                    va = va_ps.tile([C, CHUNK], F32)
                    z = z_ps.tile([1, CHUNK], F32)
                    pts = {}
                    for jt in range(NJT + SKEW):
                        if jt < NJT:
                            jsl = slice(jt * 128, (jt + 1) * 128)
                            et = et_ps.tile([C, CHUNK], F32)
                            for h in range(CHUNK // 512):
                                hs = slice(h * 512, (h + 1) * 512)
                                ihs = slice(i0 + h * 512, i0 + (h + 1) * 512)
                                nc.tensor.matmul(et[:, hs], lhsT=k_sb[:, jsl],
                                                 rhs=q_sb[:, ihs],
                                                 start=True, stop=True)
                            pt = pt_pool.tile([C, CHUNK], BF16)
                            nc.scalar.activation(pt, et, AF.Exp)
                            pts[jt] = pt
                        lag = jt - SKEW
                        if lag >= 0:
                            pjsl = slice(lag * 128, (lag + 1) * 128)
                            pt = pts[lag]
                            for h in range(CHUNK // 512):
                                hs = slice(h * 512, (h + 1) * 512)
                                nc.tensor.matmul(va[:, hs], lhsT=vt_sb[:, pjsl],
                                                 rhs=pt[:, hs],
                                                 start=(lag == 0),
                                                 stop=(lag == NJT - 1))
                            # quad-summed Z: DVE pre-sums 4 pt tiles, one
                            # M=1 matmul per 512-half per quad
                            if lag % QUAD == QUAD - 1:
                                qd = lag // QUAD
                                p0, p1, p2, p3 = (pts.pop(lag - 3), pts.pop(lag - 2),
                                                  pts.pop(lag - 1), pts.pop(lag))
                                sa = qs_pool.tile([C, CHUNK], BF16, tag="sa")
                                nc.vector.tensor_add(sa, p0, p1)
                                sb_ = qs_pool.tile([C, CHUNK], BF16, tag="sb")
                                nc.vector.tensor_add(sb_, p2, p3)
                                sab = qs_pool.tile([C, CHUNK], BF16, tag="sab")
                                nc.vector.tensor_add(sab, sa, sb_)
                                for h in range(CHUNK // 512):
                                    hs = slice(h * 512, (h + 1) * 512)
                                    nc.tensor.matmul(z[0:1, hs], lhsT=ones_sb,
                                                     rhs=sab[:, hs],
                                                     start=(qd == 0),
                                                     stop=(qd == NJT // QUAD - 1))

                    # free the va PSUM bank quickly, then normalize from SBUF
                    va_sb = nrm.tile([C, CHUNK], F32, tag="va_sb")
                    nc.vector.tensor_copy(va_sb, va)
                    z_sb = nrm.tile([1, CHUNK], F32, tag="z_sb")
                    nc.vector.tensor_copy(z_sb, z)
                    zd = dramp.tile([1, CHUNK], F32)
                    nc.sync.dma_start(zd, z_sb)
                    zb = nrm.tile([C, CHUNK], F32, tag="zb")
                    nc.sync.dma_start(zb, zd.to_broadcast([C, CHUNK]))
                    rzb = nrm.tile([C, CHUNK], F32, tag="rzb")
                    nc.vector.reciprocal(rzb, zb)
                    t = nrm.tile([C, CHUNK], F32, tag="t")
                    nc.vector.tensor_mul(t, va_sb, rzb)
                    o = outb.tile([C, CHUNK], F32)
                    nc.vector.scalar_tensor_tensor(
                        o, in0=t, scalar=gam_sb, in1=pose_sb[:, isl],
                        op0=ALU.mult, op1=ALU.add)
                    nc.sync.dma_start(out_d[:, isl], o)

    nc.compile()
    return nc


def _get_nc():
    if "nc" not in _CACHE:
        _CACHE["nc"] = _build()
    return _CACHE["nc"]


def kernel(pose_f, id_f, Wq, bq, Wk, bk, Wv, bv, gamma, **run_kwargs):
    pose_f = np.asarray(pose_f, dtype=np.float32)
    id_f = np.asarray(id_f, dtype=np.float32)
    Wq = np.asarray(Wq, dtype=np.float32)
    Wk = np.asarray(Wk, dtype=np.float32)
    Wv = np.asarray(Wv, dtype=np.float32)
    bq = np.asarray(bq, dtype=np.float32)
    bk = np.asarray(bk, dtype=np.float32)
    bv = np.asarray(bv, dtype=np.float32)
    g = float(np.asarray(gamma, dtype=np.float32).reshape(-1)[0])

    bf = ml_dtypes.bfloat16
    wt = np.concatenate([Wq.T, Wk.T, Wv.T], axis=1).astype(bf)  # [C_in, 3C]
    posebf = pose_f.astype(bf)
    idbf = id_f.astype(bf)
    bq_c = np.ascontiguousarray(bq.reshape(C, 1))
    bk_c = np.ascontiguousarray(bk.reshape(C, 1))
    bfin = np.ascontiguousarray((g * bv).reshape(C, 1).astype(np.float32))
    gam = np.full((C, 1), g, dtype=np.float32)

    in_maps = []
    for b in range(B):
        in_maps.append({
            "pose": pose_f[b],
            "posebf": posebf[b],
            "idbf": idbf[b],
            "wt": wt,
            "bq": bq_c,
            "bk": bk_c,
            "bfin": bfin,
            "gam": gam,
        })

    nc = _get_nc()
    res = run_bass_kernel_spmd(nc, in_maps, core_ids=list(range(B)), **run_kwargs)
    out = np.stack([res.results[b]["out"] for b in range(B)], axis=0)
    if run_kwargs:
        _CACHE["last_result"] = res
    return out


# revision 7
# speedup vs baseline: 1.4391x; 1.1452x over previous
"""Trainium2 Bass kernel for nn_CGPBlock (attention block with 1x1-conv QKV).

Reference computation (per batch b):
    q = Wq @ pose + bq; k = Wk @ id + bk; v = Wv @ pose + bv     # [C, L]
    energy[i, j] = sum_c q[c, i] k[c, j]                          # [L, L]
    attn = softmax_j(energy)
    va[c, i] = sum_j v[c, j] attn[i, j]
    out = pose + gamma * va

Sharding: data-parallel over batch, B=8 batches -> 8 NeuronCores (SPMD, no
collectives). Per core: C=128 fits the partition dim exactly, L=4096.

Device algorithm (per core, matmuls bf16 with fp32 PSUM accumulate):
  - q = WqT.T @ pose_bf + bq  -> q_sb bf16 [C, L]
  - k = WkT.T @ id_bf + bk    -> k_sb bf16 [C, L]
  - vT[j, c] = pose_bf[:, jt].T @ WvT -> vt_sb bf16 (v bias folded into the
    output residual, since attn rows sum to 1: va/Z + bv).
  - For each i-chunk (1024 cols), accumulate over 32 j-tiles:
      eT[j, i] = k_jt.T @ q_chunk         (PSUM, 2 x N=512 matmuls)
      pT = exp(eT)                        (one ACT op per j-tile; no max-sub:
                                           |E| < 32 so fp32 exp is safe, and
                                           softmax is shift-invariant)
      va[c, i] += vt_jt.T @ pT            (PSUM accumulate)
      Z[1, i]  += ones.T @ (pT quadsum)   (DVE pre-sums 4 j-tiles, then one
                                           M=1 matmul per quad — softmax
                                           column sums in the [j,i] layout
                                           without any transposes)
    then out = pose' + gamma * va * (1/Z), pose' = pose + gamma*bv.
  - Z is broadcast across partitions via a tiny DRAM round-trip; reciprocal
    runs on the broadcast [C, chunk] tile (1-partition DVE ops are slow).

Emission order software-pipelines PE vs ACT (va matmuls run SKEW j-tiles
behind the energy matmuls) because each engine executes in program order.
"""

import numpy as np
import ml_dtypes

import concourse.bacc as bacc
import concourse.tile as tile
from concourse import mybir
from concourse.bass_utils import run_bass_kernel_spmd

F32 = mybir.dt.float32
BF16 = mybir.dt.bfloat16
AF = mybir.ActivationFunctionType
ALU = mybir.AluOpType

B, C, L = 8, 128, 4096
CHUNK = 1024                # i-chunk width
NCH = L // CHUNK            # 4 chunks
NJT = L // 128              # 32 j-tiles
QUAD = 4                    # j-tiles pre-summed per Z matmul
SKEW = 6                    # software pipeline depth (PE runs ahead of ACT)

_CACHE = {}


def _build():
    nc = bacc.Bacc("TRN2", target_bir_lowering=False, debug=False, num_devices=B)

    pose_d = nc.dram_tensor("pose", [C, L], F32, kind="ExternalInput").ap()
    posebf_d = nc.dram_tensor("posebf", [C, L], BF16, kind="ExternalInput").ap()
    idbf_d = nc.dram_tensor("idbf", [C, L], BF16, kind="ExternalInput").ap()
    wt_d = nc.dram_tensor("wt", [C, 3 * C], BF16, kind="ExternalInput").ap()
    bq_d = nc.dram_tensor("bq", [C, 1], F32, kind="ExternalInput").ap()
    bk_d = nc.dram_tensor("bk", [C, 1], F32, kind="ExternalInput").ap()
    bfin_d = nc.dram_tensor("bfin", [C, 1], F32, kind="ExternalInput").ap()
    gam_d = nc.dram_tensor("gam", [C, 1], F32, kind="ExternalInput").ap()
    out_d = nc.dram_tensor("out", [C, L], F32, kind="ExternalOutput").ap()

    with tile.TileContext(nc) as tc:
        with tc.tile_pool(name="res", bufs=1) as res:
            wt_sb = res.tile([C, 3 * C], BF16)
            nc.gpsimd.dma_start(wt_sb, wt_d)
            bq_sb = res.tile([C, 1], F32)
            nc.gpsimd.dma_start(bq_sb, bq_d)
            bk_sb = res.tile([C, 1], F32)
            nc.gpsimd.dma_start(bk_sb, bk_d)
            bfin_sb = res.tile([C, 1], F32)
            nc.gpsimd.dma_start(bfin_sb, bfin_d)
            gam_sb = res.tile([C, 1], F32)
            nc.gpsimd.dma_start(gam_sb, gam_d)
            ones_sb = res.tile([C, 1], BF16)
            nc.vector.memset(ones_sb, 1.0)

            pose_sb = res.tile([C, L], F32)
            posebf_sb = res.tile([C, L], BF16)
            idbf_sb = res.tile([C, L], BF16)
            q_sb = res.tile([C, L], BF16)
            k_sb = res.tile([C, L], BF16)
            vt_sb = res.tile([C, L], BF16)   # [j (partition), jt*128 + c]

            # Chunked input DMA across two queues so QKV matmuls start early.
            for ch in range(NCH):
                sl = slice(ch * CHUNK, (ch + 1) * CHUNK)
                nc.sync.dma_start(idbf_sb[:, sl], idbf_d[:, sl])
                nc.gpsimd.dma_start(posebf_sb[:, sl], posebf_d[:, sl])
            for ch in range(NCH):
                sl = slice(ch * CHUNK, (ch + 1) * CHUNK)
                eng = nc.sync if ch % 2 == 0 else nc.gpsimd
                eng.dma_start(pose_sb[:, sl], pose_d[:, sl])

            wqT = wt_sb[:, 0:C]
            wkT = wt_sb[:, C:2 * C]
            wvT = wt_sb[:, 2 * C:3 * C]

            # ---- QKV convs (1x1 = channel-mixing matmuls) ----
            with tc.tile_pool(name="conv_ps", bufs=2, space="PSUM") as conv_ps:
                for ch in range(NCH):
                    kp = conv_ps.tile([C, CHUNK], F32, tag="kp")
                    qp = conv_ps.tile([C, CHUNK], F32, tag="qp")
                    for h in range(CHUNK // 512):
                        sl = slice(ch * CHUNK + h * 512, ch * CHUNK + (h + 1) * 512)
                        hs = slice(h * 512, (h + 1) * 512)
                        nc.tensor.matmul(kp[:, hs], lhsT=wkT, rhs=idbf_sb[:, sl],
                                         start=True, stop=True)
                        nc.tensor.matmul(qp[:, hs], lhsT=wqT, rhs=posebf_sb[:, sl],
                                         start=True, stop=True)
                    csl = slice(ch * CHUNK, (ch + 1) * CHUNK)
                    nc.vector.tensor_scalar_add(k_sb[:, csl], kp, bk_sb)
                    nc.vector.tensor_scalar_add(q_sb[:, csl], qp, bq_sb)
                for jt in range(NJT):
                    jsl = slice(jt * 128, (jt + 1) * 128)
                    # vp rides the qp slots (tag share) to stay within PSUM
                    vp = conv_ps.tile([C, CHUNK], F32, tag="qp", name="vp")
                    nc.tensor.matmul(vp[:, 0:128], lhsT=posebf_sb[:, jsl],
                                     rhs=wvT, start=True, stop=True)
                    nc.vector.tensor_copy(vt_sb[:, jsl], vp[:, 0:128])

            # pose' = pose + gamma*bv (per-partition const), reused as the
            # residual; deferred so it doesn't gate the conv DVE work
            nc.vector.tensor_scalar_add(pose_sb, pose_sb, bfin_sb)

            # ---- attention ----
            with (
                tc.tile_pool(name="et_ps", bufs=2, space="PSUM") as et_ps,
                tc.tile_pool(name="va_ps", bufs=1, space="PSUM") as va_ps,
                tc.tile_pool(name="z_ps", bufs=1, space="PSUM") as z_ps,
                tc.tile_pool(name="pt_sb", bufs=SKEW + QUAD + 2) as pt_pool,
                tc.tile_pool(name="qs_sb", bufs=2) as qs_pool,
                tc.tile_pool(name="nrm", bufs=2) as nrm,
                tc.tile_pool(name="outb", bufs=2) as outb,
                tc.tile_pool(name="dramp", bufs=2, space="DRAM") as dramp,
            ):
                for ch in range(NCH):
                    i0 = ch * CHUNK
                    isl = slice(i0, i0 + CHUNK)
                    va = va_ps.tile([C, CHUNK], F32)
                    z = z_ps.tile([1, CHUNK], F32)
                    pts = {}
                    for jt in range(NJT + SKEW):
                        if jt < NJT:
                            jsl = slice(jt * 128, (jt + 1) * 128)
                            et = et_ps.tile([C, CHUNK], F32)
                            for h in range(CHUNK // 512):
                                hs = slice(h * 512, (h + 1) * 512)
                                ihs = slice(i0 + h * 512, i0 + (h + 1) * 512)
                                nc.tensor.matmul(et[:, hs], lhsT=k_sb[:, jsl],
                                                 rhs=q_sb[:, ihs],
                                                 start=True, stop=True)
                            pt = pt_pool.tile([C, CHUNK], BF16)
                            nc.scalar.activation(pt, et, AF.Exp)
                            pts[jt] = pt
                        lag = jt - SKEW
                        if lag >= 0:
                            pjsl = slice(lag * 128, (lag + 1) * 128)
                            pt = pts[lag]
                            for h in range(CHUNK // 512):
                                hs = slice(h * 512, (h + 1) * 512)
                                nc.tensor.matmul(va[:, hs], lhsT=vt_sb[:, pjsl],
                                                 rhs=pt[:, hs],
                                                 start=(lag == 0),
                                                 stop=(lag == NJT - 1))
                            # quad-summed Z: DVE pre-sums 4 pt tiles, one
                            # M=1 matmul per 512-half per quad
                            if lag % QUAD == QUAD - 1:
                                qd = lag // QUAD
                                p0, p1, p2, p3 = (pts.pop(lag - 3), pts.pop(lag - 2),
                                                  pts.pop(lag - 1), pts.pop(lag))
                                sa = qs_pool.tile([C, CHUNK], BF16, tag="sa")
                                nc.vector.tensor_add(sa, p0, p1)
                                sb_ = qs_pool.tile([C, CHUNK], BF16, tag="sb")
                                nc.vector.tensor_add(sb_, p2, p3)
                                sab = qs_pool.tile([C, CHUNK], BF16, tag="sab")
                                nc.vector.tensor_add(sab, sa, sb_)
                                for h in range(CHUNK // 512):
                                    hs = slice(h * 512, (h + 1) * 512)
                                    nc.tensor.matmul(z[0:1, hs], lhsT=ones_sb,
                                                     rhs=sab[:, hs],
                                                     start=(qd == 0),
                                                     stop=(qd == NJT // QUAD - 1))

                    # free the va PSUM bank quickly, then normalize from SBUF
                    va_sb = nrm.tile([C, CHUNK], F32, tag="va_sb")
                    nc.vector.tensor_copy(va_sb, va)
                    rz = nrm.tile([1, CHUNK], F32, tag="rz")
                    nc.vector.reciprocal_approx_fast(rz, z)
                    zd = dramp.tile([1, CHUNK], F32)
                    nc.sync.dma_start(zd, rz)
                    rzb = nrm.tile([C, CHUNK], F32, tag="rzb")
                    nc.sync.dma_start(rzb, zd.to_broadcast([C, CHUNK]))
                    t = nrm.tile([C, CHUNK], F32, tag="t")
                    nc.vector.tensor_mul(t, va_sb, rzb)
                    o = outb.tile([C, CHUNK], F32)
                    nc.vector.scalar_tensor_tensor(
                        o, in0=t, scalar=gam_sb, in1=pose_sb[:, isl],
                        op0=ALU.mult, op1=ALU.add)
                    nc.sync.dma_start(out_d[:, isl], o)

    nc.compile()
    return nc


def _get_nc():
    if "nc" not in _CACHE:
        _CACHE["nc"] = _build()
    return _CACHE["nc"]


def kernel(pose_f, id_f, Wq, bq, Wk, bk, Wv, bv, gamma, **run_kwargs):
    pose_f = np.asarray(pose_f, dtype=np.float32)
    id_f = np.asarray(id_f, dtype=np.float32)
    Wq = np.asarray(Wq, dtype=np.float32)
    Wk = np.asarray(Wk, dtype=np.float32)
    Wv = np.asarray(Wv, dtype=np.float32)
    bq = np.asarray(bq, dtype=np.float32)
    bk = np.asarray(bk, dtype=np.float32)
    bv = np.asarray(bv, dtype=np.float32)
    g = float(np.asarray(gamma, dtype=np.float32).reshape(-1)[0])

    bf = ml_dtypes.bfloat16
    wt = np.concatenate([Wq.T, Wk.T, Wv.T], axis=1).astype(bf)  # [C_in, 3C]
    posebf = pose_f.astype(bf)
    idbf = id_f.astype(bf)
    bq_c = np.ascontiguousarray(bq.reshape(C, 1))
    bk_c = np.ascontiguousarray(bk.reshape(C, 1))
    bfin = np.ascontiguousarray((g * bv).reshape(C, 1).astype(np.float32))
    gam = np.full((C, 1), g, dtype=np.float32)

    in_maps = []
    for b in range(B):
        in_maps.append({
            "pose": pose_f[b],
            "posebf": posebf[b],
            "idbf": idbf[b],
            "wt": wt,
            "bq": bq_c,
            "bk": bk_c,
            "bfin": bfin,
            "gam": gam,
        })

    nc = _get_nc()
    res = run_bass_kernel_spmd(nc, in_maps, core_ids=list(range(B)), **run_kwargs)
    out = np.stack([res.results[b]["out"] for b in range(B)], axis=0)
    if run_kwargs:
        _CACHE["last_result"] = res
    return out


# revision 11
# speedup vs baseline: 1.4603x; 1.0148x over previous
"""Trainium2 Bass kernel for nn_CGPBlock (attention block with 1x1-conv QKV).

Reference computation (per batch b):
    q = Wq @ pose + bq; k = Wk @ id + bk; v = Wv @ pose + bv     # [C, L]
    energy[i, j] = sum_c q[c, i] k[c, j]                          # [L, L]
    attn = softmax_j(energy)
    va[c, i] = sum_j v[c, j] attn[i, j]
    out = pose + gamma * va

Sharding: data-parallel over batch, B=8 batches -> 8 NeuronCores (SPMD, no
collectives). Per core: C=128 fits the partition dim exactly, L=4096.

Device algorithm (per core, matmuls bf16 with fp32 PSUM accumulate):
  - q = WqT.T @ pose_bf + bq  -> q_sb bf16 [C, L]
  - k = WkT.T @ id_bf + bk    -> k_sb bf16 [C, L]
  - vT[j, c] = pose_bf[:, jt].T @ WvT -> vt_sb bf16 (v bias folded into the
    output residual, since attn rows sum to 1: va/Z + bv).
  - For each i-chunk (1024 cols), accumulate over 32 j-tiles:
      eT[j, i] = k_jt.T @ q_chunk         (PSUM, 2 x N=512 matmuls)
      pT = exp(eT)                        (one ACT op per j-tile; no max-sub:
                                           |E| < 32 so fp32 exp is safe, and
                                           softmax is shift-invariant)
      va[c, i] += vt_jt.T @ pT            (PSUM accumulate)
      Z[1, i]  += ones.T @ (pT quadsum)   (DVE pre-sums 4 j-tiles, then one
                                           M=1 matmul per quad — softmax
                                           column sums in the [j,i] layout
                                           without any transposes)
    then out = pose' + gamma * va * (1/Z), pose' = pose + gamma*bv.
  - Z is broadcast across partitions via a tiny DRAM round-trip; reciprocal
    runs on the broadcast [C, chunk] tile (1-partition DVE ops are slow).

Emission order software-pipelines PE vs ACT (va matmuls run SKEW j-tiles
behind the energy matmuls) because each engine executes in program order.
"""

import numpy as np
import ml_dtypes

import concourse.bacc as bacc
import concourse.tile as tile
from concourse import mybir
from concourse.bass_utils import run_bass_kernel_spmd

F32 = mybir.dt.float32
BF16 = mybir.dt.bfloat16
AF = mybir.ActivationFunctionType
ALU = mybir.AluOpType

B, C, L = 8, 128, 4096
CHUNK = 1024                # i-chunk width
NCH = L // CHUNK            # 4 chunks
NJT = L // 128              # 32 j-tiles
QUAD = 4                    # j-tiles pre-summed per Z matmul
SKEW = 6                    # software pipeline depth (PE runs ahead of ACT)

_CACHE = {}


def _build():
    nc = bacc.Bacc("TRN2", target_bir_lowering=False, debug=False, num_devices=B)

    pose_d = nc.dram_tensor("pose", [C, L], F32, kind="ExternalInput").ap()
    posebf_d = nc.dram_tensor("posebf", [C, L], BF16, kind="ExternalInput").ap()
    idbf_d = nc.dram_tensor("idbf", [C, L], BF16, kind="ExternalInput").ap()
    wt_d = nc.dram_tensor("wt", [C, 3 * C], BF16, kind="ExternalInput").ap()
    bq_d = nc.dram_tensor("bq", [C, 1], F32, kind="ExternalInput").ap()
    bk_d = nc.dram_tensor("bk", [C, 1], F32, kind="ExternalInput").ap()
    bfin_d = nc.dram_tensor("bfin", [C, 1], F32, kind="ExternalInput").ap()
    gam_d = nc.dram_tensor("gam", [C, 1], F32, kind="ExternalInput").ap()
    out_d = nc.dram_tensor("out", [C, L], F32, kind="ExternalOutput").ap()

    with tile.TileContext(nc) as tc:
        with tc.tile_pool(name="res", bufs=1) as res:
            bq_sb = res.tile([C, 1], F32)
            nc.sync.dma_start(bq_sb, bq_d)
            bk_sb = res.tile([C, 1], F32)
            nc.sync.dma_start(bk_sb, bk_d)
            wt_sb = res.tile([C, 3 * C], BF16)
            nc.sync.dma_start(wt_sb, wt_d)
            bfin_sb = res.tile([C, 1], F32)
            nc.gpsimd.dma_start(bfin_sb, bfin_d)
            gam_sb = res.tile([C, 1], F32)
            nc.gpsimd.dma_start(gam_sb, gam_d)
            ones_sb = res.tile([C, 1], BF16)
            nc.vector.memset(ones_sb, 1.0)
            onesr_sb = res.tile([1, C], F32)
            nc.vector.memset(onesr_sb, 1.0)

            pose_sb = res.tile([C, L], F32)
            posebf_sb = res.tile([C, L], BF16)
            idbf_sb = res.tile([C, L], BF16)
            q_sb = res.tile([C, L], BF16)
            k_sb = res.tile([C, L], BF16)
            v_sb = res.tile([C, L], BF16)
            vt_sb = res.tile([C, L], BF16)   # [j (partition), jt*128 + c]

            # Chunked input DMA so QKV matmuls start early; pose (residual,
            # needed late) goes on the gpsimd SW queue.
            for ch in range(NCH):
                sl = slice(ch * CHUNK, (ch + 1) * CHUNK)
                nc.sync.dma_start(idbf_sb[:, sl], idbf_d[:, sl])
                nc.sync.dma_start(posebf_sb[:, sl], posebf_d[:, sl])
            for ch in range(NCH):
                sl = slice(ch * CHUNK, (ch + 1) * CHUNK)
                nc.gpsimd.dma_start(pose_sb[:, sl], pose_d[:, sl])

            wqT = wt_sb[:, 0:C]
            wkT = wt_sb[:, C:2 * C]
            wvT = wt_sb[:, 2 * C:3 * C]

            # ---- QKV convs (1x1 = channel-mixing matmuls) ----
            with tc.tile_pool(name="conv_ps", bufs=2, space="PSUM") as conv_ps:
                for ch in range(NCH):
                    kp = conv_ps.tile([C, CHUNK], F32, tag="kp")
                    qp = conv_ps.tile([C, CHUNK], F32, tag="qp")
                    for h in range(CHUNK // 512):
                        sl = slice(ch * CHUNK + h * 512, ch * CHUNK + (h + 1) * 512)
                        hs = slice(h * 512, (h + 1) * 512)
                        nc.tensor.matmul(kp[:, hs], lhsT=wkT, rhs=idbf_sb[:, sl],
                                         start=True, stop=True)
                        nc.tensor.matmul(qp[:, hs], lhsT=wqT, rhs=posebf_sb[:, sl],
                                         start=True, stop=True)
                    csl = slice(ch * CHUNK, (ch + 1) * CHUNK)
                    nc.vector.tensor_scalar_add(k_sb[:, csl], kp, bk_sb)
                    nc.vector.tensor_scalar_add(q_sb[:, csl], qp, bq_sb)
                for ch in range(NCH):
                    vp = conv_ps.tile([C, CHUNK], F32,
                                      tag=("kp" if ch % 2 == 0 else "qp"),
                                      name="vp")
                    for h in range(CHUNK // 512):
                        sl = slice(ch * CHUNK + h * 512, ch * CHUNK + (h + 1) * 512)
                        hs = slice(h * 512, (h + 1) * 512)
                        nc.tensor.matmul(vp[:, hs], lhsT=wvT,
                                         rhs=posebf_sb[:, sl],
                                         start=True, stop=True)
                    # no bias: v's bias is folded into the residual (attn rows
                    # sum to 1 after normalization)
                    csl = slice(ch * CHUNK, (ch + 1) * CHUNK)
                    nc.vector.tensor_copy(v_sb[:, csl], vp)

            # vT tiles via DMA xbar transpose (off the PE critical path)
            for jt in range(NJT):
                jsl = slice(jt * 128, (jt + 1) * 128)
                nc.sync.dma_start(vt_sb[:, jsl], v_sb[:, jsl], transpose=True)

            # pose' = pose + gamma*bv (per-partition const), reused as the
            # residual; deferred so it doesn't gate the conv DVE work
            nc.vector.tensor_scalar_add(pose_sb, pose_sb, bfin_sb)

            # ---- attention ----
            with (
                tc.tile_pool(name="et_ps", bufs=2, space="PSUM") as et_ps,
                tc.tile_pool(name="va_ps", bufs=1, space="PSUM") as va_ps,
                tc.tile_pool(name="z_ps", bufs=1, space="PSUM") as z_ps,
                tc.tile_pool(name="pt_sb", bufs=SKEW + QUAD + 2) as pt_pool,
                tc.tile_pool(name="qs_sb", bufs=2) as qs_pool,
                tc.tile_pool(name="nrm", bufs=2) as nrm,
                tc.tile_pool(name="outb", bufs=2) as outb,
                tc.tile_pool(name="dramp", bufs=2, space="DRAM") as dramp,
            ):
                for ch in range(NCH):
                    i0 = ch * CHUNK
                    isl = slice(i0, i0 + CHUNK)
                    va = va_ps.tile([C, CHUNK], F32)
                    z = z_ps.tile([1, CHUNK], F32)
                    pts = {}
                    for jt in range(NJT + SKEW):
                        if jt < NJT:
                            jsl = slice(jt * 128, (jt + 1) * 128)
                            et = et_ps.tile([C, CHUNK], F32)
                            for h in range(CHUNK // 512):
                                hs = slice(h * 512, (h + 1) * 512)
                                ihs = slice(i0 + h * 512, i0 + (h + 1) * 512)
                                nc.tensor.matmul(et[:, hs], lhsT=k_sb[:, jsl],
                                                 rhs=q_sb[:, ihs],
                                                 start=True, stop=True)
                            pt = pt_pool.tile([C, CHUNK], BF16)
                            nc.scalar.activation(pt, et, AF.Exp)
                            pts[jt] = pt
                        lag = jt - SKEW
                        if lag >= 0:
                            pjsl = slice(lag * 128, (lag + 1) * 128)
                            pt = pts[lag]
                            for h in range(CHUNK // 512):
                                hs = slice(h * 512, (h + 1) * 512)
                                nc.tensor.matmul(va[:, hs], lhsT=vt_sb[:, pjsl],
                                                 rhs=pt[:, hs],
                                                 start=(lag == 0),
                                                 stop=(lag == NJT - 1))
                            # quad-summed Z: DVE pre-sums 4 pt tiles, one
                            # M=1 matmul per 512-half per quad
                            if lag % QUAD == QUAD - 1:
                                qd = lag // QUAD
                                p0, p1, p2, p3 = (pts.pop(lag - 3), pts.pop(lag - 2),
                                                  pts.pop(lag - 1), pts.pop(lag))
                                sa = qs_pool.tile([C, CHUNK], BF16, tag="sa")
                                nc.vector.tensor_add(sa, p0, p1)
                                sb_ = qs_pool.tile([C, CHUNK], BF16, tag="sb")
                                nc.vector.tensor_add(sb_, p2, p3)
                                sab = qs_pool.tile([C, CHUNK], BF16, tag="sab")
                                nc.vector.tensor_add(sab, sa, sb_)
                                for h in range(CHUNK // 512):
                                    hs = slice(h * 512, (h + 1) * 512)
                                    nc.tensor.matmul(z[0:1, hs], lhsT=ones_sb,
                                                     rhs=sab[:, hs],
                                                     start=(qd == 0),
                                                     stop=(qd == NJT // QUAD - 1))

                    # free the va PSUM bank quickly, then normalize from SBUF
                    va_sb = nrm.tile([C, CHUNK], F32, tag="va_sb")
                    nc.vector.tensor_copy(va_sb, va)
                    rz = nrm.tile([1, CHUNK], F32, tag="rz")
                    nc.vector.reciprocal_approx_fast(rz, z)
                    if ch < NCH - 1:
                        # broadcast 1/Z across partitions via DRAM round-trip
                        # (latency hidden under the next chunk's compute)
                        zd = dramp.tile([1, CHUNK], F32)
                        nc.sync.dma_start(zd, rz)
                        rzb = nrm.tile([C, CHUNK], F32, tag="rzb")
                        nc.sync.dma_start(rzb, zd.to_broadcast([C, CHUNK]))
                    else:
                        # last chunk: nothing left to hide the DMA latency
                        # under — broadcast on the (now idle) PE instead
                        rzb = et_ps.tile([C, CHUNK], F32, tag="et", name="rzb_ps")
                        for h in range(CHUNK // 512):
                            hs = slice(h * 512, (h + 1) * 512)
                            nc.tensor.matmul(rzb[:, hs], lhsT=onesr_sb,
                                             rhs=rz[0:1, hs],
                                             start=True, stop=True)
                    t = nrm.tile([C, CHUNK], F32, tag="t")
                    nc.vector.tensor_mul(t, va_sb, rzb)
                    o = outb.tile([C, CHUNK], F32)
                    nc.vector.scalar_tensor_tensor(
                        o, in0=t, scalar=gam_sb, in1=pose_sb[:, isl],
                        op0=ALU.mult, op1=ALU.add)
                    nc.sync.dma_start(out_d[:, isl], o)

    nc.compile()
    return nc


def _get_nc():
    if "nc" not in _CACHE:
        _CACHE["nc"] = _build()
    return _CACHE["nc"]


def kernel(pose_f, id_f, Wq, bq, Wk, bk, Wv, bv, gamma, **run_kwargs):
    pose_f = np.asarray(pose_f, dtype=np.float32)
    id_f = np.asarray(id_f, dtype=np.float32)
    Wq = np.asarray(Wq, dtype=np.float32)
    Wk = np.asarray(Wk, dtype=np.float32)
    Wv = np.asarray(Wv, dtype=np.float32)
    bq = np.asarray(bq, dtype=np.float32)
    bk = np.asarray(bk, dtype=np.float32)
    bv = np.asarray(bv, dtype=np.float32)
    g = float(np.asarray(gamma, dtype=np.float32).reshape(-1)[0])

    bf = ml_dtypes.bfloat16
    wt = np.concatenate([Wq.T, Wk.T, Wv.T], axis=1).astype(bf)  # [C_in, 3C]
    posebf = pose_f.astype(bf)
    idbf = id_f.astype(bf)
    bq_c = np.ascontiguousarray(bq.reshape(C, 1))
    bk_c = np.ascontiguousarray(bk.reshape(C, 1))
    bfin = np.ascontiguousarray((g * bv).reshape(C, 1).astype(np.float32))
    gam = np.full((C, 1), g, dtype=np.float32)

    in_maps = []
    for b in range(B):
        in_maps.append({
            "pose": pose_f[b],
            "posebf": posebf[b],
            "idbf": idbf[b],
            "wt": wt,
            "bq": bq_c,
            "bk": bk_c,
            "bfin": bfin,
            "gam": gam,
        })

    nc = _get_nc()
    res = run_bass_kernel_spmd(nc, in_maps, core_ids=list(range(B)), **run_kwargs)
    out = np.stack([res.results[b]["out"] for b in range(B)], axis=0)
    if run_kwargs:
        _CACHE["last_result"] = res
    return out
